# revision 1
# baseline (speedup 1.0000x reference)
"""Trainium2 Bass kernel for DiagonalUpsample (checkerboard 2x interleave).

  out[2i,   2j  ] = d[i,j];  out[2i,   2j+1] = u[i,j]
  out[2i+1, 2j  ] = u[i,j];  out[2i+1, 2j+1] = d[i,j]

Sharding: pure data parallel over the batch dim (16 -> 2 per core x 8 cores).

Per-core layout: the (2,3,512,512) shard is viewed as 3072 contiguous input
rows of 512 f32.  Each SBUF tile packs K consecutive input rows per partition,
so the corresponding 2K output rows per partition are one contiguous run of
bytes in HBM -> fully contiguous load/store DMAs.  The 4-byte checkerboard
interleave itself runs on the vector engine as 4 strided tensor_copys per tile
(fp32 2x mode).  All inputs are loaded in one read run before any store
(single HWDGE FIFO ring) so HBM never pays read/write turnaround mid-kernel.
"""

import numpy as np

import concourse.bass as bass
import concourse.tile as tile
from concourse import bacc, mybir
from concourse.bass_utils import run_bass_kernel_spmd
from concourse.tile import add_dep_helper

B, C, H, W = 16, 3, 512, 512
N_CORES = 8
B_LOC = B // N_CORES           # 2 batches per core
ROWS = B_LOC * C * H           # 3072 input rows per core
P = 128                        # SBUF partitions
K = 6                          # input rows per partition per tile
TILE_ROWS = P * K              # 768 input rows per tile
N_TILES = ROWS // TILE_ROWS    # 4 tiles per core
FP32 = mybir.dt.float32

_nc_cache = []

# test-harness knobs (ignored in normal grading use)
TRACE = False
LAST_RESULT = None


def _build_nc() -> bass.Bass:
    nc = bacc.Bacc("TRN2", debug=False)
    up = nc.dram_tensor("up", [N_TILES, P, K * W], FP32, kind="ExternalInput")
    down = nc.dram_tensor("down", [N_TILES, P, K * W], FP32, kind="ExternalInput")
    out = nc.dram_tensor("out", [N_TILES, P, K * 4 * W], FP32, kind="ExternalOutput")

    with tile.TileContext(nc) as tc:
        with (
            tc.tile_pool(name="inp", bufs=N_TILES) as inp,
            tc.tile_pool(name="outp", bufs=2) as outp,
        ):
            # one long read run (all input loads), then one long write run,
            # all on the sync HWDGE ring (FIFO): avoids HBM read/write
            # turnaround penalties mid-kernel (~17% measured).
            us, ds = [], []
            last_load = None
            for t in range(N_TILES):
                u = inp.tile([P, K * W], FP32, tag="u")
                nc.sync.dma_start(u[:], up[t])
                d = inp.tile([P, K * W], FP32, tag="d")
                last_load = nc.sync.dma_start(d[:], down[t])
                us.append(u)
                ds.append(d)
            for t in range(N_TILES):
                o = outp.tile([P, K * 4 * W], FP32, tag="o")
                # per-partition layout: k (input row) x r (out-row
                # parity) x w (out col pair) x c (out col parity)
                ov = o.rearrange("p (k r w c) -> p k r c w", k=K, r=2, w=W, c=2)
                uv = us[t].rearrange("p (k w) -> p k w", k=K)
                dv = ds[t].rearrange("p (k w) -> p k w", k=K)
                nc.vector.tensor_copy(ov[:, :, 0, 0, :], dv[:])
                nc.vector.tensor_copy(ov[:, :, 0, 1, :], uv[:])
                nc.vector.tensor_copy(ov[:, :, 1, 0, :], uv[:])
                nc.vector.tensor_copy(ov[:, :, 1, 1, :], dv[:])
                store = nc.sync.dma_start(out[t], o[:])
                # pin phase order: no store may be scheduled before the
                # read run completes (direction mixing costs ~20% HBM bw)
                add_dep_helper(store.ins, last_load.ins, sync=False,
                               reason="write phase after read phase")
    nc.compile()
    return nc


def _get_nc() -> bass.Bass:
    if not _nc_cache:
        _nc_cache.append(_build_nc())
    return _nc_cache[0]


def kernel(up_diagonal: np.ndarray, down_diagonal: np.ndarray) -> np.ndarray:
    up_diagonal = np.ascontiguousarray(np.asarray(up_diagonal, dtype=np.float32))
    down_diagonal = np.ascontiguousarray(np.asarray(down_diagonal, dtype=np.float32))
    assert up_diagonal.shape == (B, C, H, W), up_diagonal.shape

    nc = _get_nc()
    in_maps = []
    for core in range(N_CORES):
        sl = slice(core * B_LOC, (core + 1) * B_LOC)
        in_maps.append(
            {
                "up": up_diagonal[sl].reshape(N_TILES, P, K * W),
                "down": down_diagonal[sl].reshape(N_TILES, P, K * W),
            }
        )

    res = run_bass_kernel_spmd(
        nc, in_maps, core_ids=list(range(N_CORES)), trace=TRACE
    )
    global LAST_RESULT
    LAST_RESULT = res
    results = res.results
    out = np.empty((B, C, 2 * H, 2 * W), dtype=np.float32)
    for core in range(N_CORES):
        sl = slice(core * B_LOC, (core + 1) * B_LOC)
        out[sl] = results[core]["out"].reshape(B_LOC, C, 2 * H, 2 * W)
    return out

